# revision 35
# baseline (speedup 1.0000x reference)
"""Multi-head attention block (dense transformer) on 8 Trainium2 NeuronCores.

Problem: x [4, 2048, 1024] f32, w_qkv [1024, 3072], w_out [1024, 1024].
  qkv = x @ w_qkv -> split (3, 16 heads, 64) -> softmax(q k^T / 8) v -> @ w_out

Sharding: tensor-parallel over heads. Core c owns heads (2c, 2c+1):
  - w_qkv columns for q/k/v of those heads -> [1024, 384]
  - w_out rows for those heads            -> [128, 1024]
  - x is pre-transposed/cast on the host to xT [1024, 8192] bf16 (the
    contraction side must sit on partitions; doing it host-side avoids 512
    on-chip PE transposes per core)
  - each core computes a full-shape partial output [8192, 1024]; the host sum
    of the 8 partials is the all-reduce.

Per-core kernel (all matmuls bf16 into fp32 PSUM):
  P1: project qT,kT,vT [128=2*64 rows, n] (scoresT-friendly layout) from xT
      tiles; PE-transpose vT back to v natural [n, 128] stored with a ones
      column per head (softmax sums).
  P2: per (batch, n_i tile of 512, n_j chunk of 128): both heads' scoresT
      [n_j=128, 512] go into one PSUM tile so the two K=64 score matmuls are
      co-ready and adjacent -> the PE row-group-packs them into one
      concurrent stream pass; one ACT exp per chunk (scale=1/8 folded in,
      no max-subtraction needed: scores ~ N(0,1)) -> bf16; av matmul
      lhsT=[v|1] (M=65) accumulates outT [64, 512] + softmax sums in row 64.
      Accumulators are evacuated to SBUF immediately (frees PSUM, keeps PE
      fed); normalization (fast reciprocal of both heads' sums + GpSimd
      partition_broadcast + DVE multiply) runs off the critical path
      -> out_stack [128, n].
  P3: partial = out_stack.T @ w_out_local, streamed to DRAM in 4-chunk
      batched stores.

The three phases are software-pipelined across batches (P2(b) interleaved
with P1(b+1) and P3(b-1)) so the PE always has dense independent work and
the HAM clock gate stays at K=8/8. PSUM budget is exactly 8 banks:
2 work (P1/P3) + 2x2 score + 2 av.
"""

import numpy as np
import ml_dtypes

import concourse.bacc as bacc
import concourse.tile as tile
from concourse import mybir, masks
from concourse.bass_utils import run_bass_kernel_spmd

F32 = mybir.dt.float32
BF16 = mybir.dt.bfloat16
EXP = mybir.ActivationFunctionType.Exp

B = 4
N = 2048
D = 1024
HEADS = 16
DH = 64
NT = B * N           # 8192 tokens
FT = D // 128        # 8 feature chunks
TT_PER_B = 4         # token tiles (512) per batch
NI = 4               # n_i tiles of 512 per batch
NJ = 16              # n_j chunks of 128 per batch
VW = 144             # v chunk: [v_A(64) | 1 | pad7 | v_B(64) | 1 | pad] (16B-aligned)

_CACHE = {}


def build():
    nc = bacc.Bacc("TRN2", target_bir_lowering=False, debug=False, num_devices=1)
    xT_d = nc.dram_tensor("xT", [D, NT], BF16, kind="ExternalInput").ap()
    wqkv_d = nc.dram_tensor("wqkv", [D, 384], BF16, kind="ExternalInput").ap()
    wout_d = nc.dram_tensor("wout", [128, D], BF16, kind="ExternalInput").ap()
    out_d = nc.dram_tensor("out", [NT, D], F32, kind="ExternalOutput").ap()
    xT_v = xT_d.rearrange("(f p) n -> f p n", p=128)

    with tile.TileContext(nc) as tc:
        with tc.tile_pool(name="const", bufs=1) as cpool, \
             tc.tile_pool(name="xt", bufs=4) as xt_pool, \
             tc.tile_pool(name="qkv", bufs=2) as qkv_pool, \
             tc.tile_pool(name="vt", bufs=3) as vt_pool, \
             tc.tile_pool(name="attn", bufs=4) as attn_pool, \
             tc.tile_pool(name="ostk", bufs=2) as ostk_pool, \
             tc.tile_pool(name="ov", bufs=4) as ov_pool, \
             tc.tile_pool(name="smol", bufs=6) as smol_pool, \
             tc.tile_pool(name="fout", bufs=2) as fout_pool, \
             tc.tile_pool(name="ps_work", bufs=2, space="PSUM") as ps_work, \
             tc.tile_pool(name="ps_score", bufs=2, space="PSUM") as ps_score, \
             tc.tile_pool(name="ps_av", bufs=2, space="PSUM") as ps_av:

            ident = cpool.tile([128, 128], BF16, tag="ident")
            masks.make_identity(nc, ident[:])

            wv = wqkv_d.rearrange("(f p) m -> p f m", p=128)
            w_sb = cpool.tile([128, FT, 384], BF16, tag="w")
            nc.sync.dma_start(w_sb[:, 0:4, :], wv[:, 0:4, :])
            nc.sync.dma_start(w_sb[:, 4:8, :], wv[:, 4:8, :])
            wout_sb = cpool.tile([128, D], BF16, tag="wout")
            nc.sync.dma_start(wout_sb[:], wout_d)

            # per-batch live tiles
            qT_t, kT_t, v_t, ostk_t, xt_t = {}, {}, {}, {}, {}

            def p1_load(b, tt):
                """Prefetch the xT chunk for token tile tt of batch b."""
                tok = slice(b * N + tt * 512, b * N + (tt + 1) * 512)
                xt_all = xt_pool.tile([128, FT, 512], BF16, tag="xt",
                                      name=f"xt{b}_{tt}")
                nc.sync.dma_start(xt_all[:], xT_v[:, :, tok].rearrange(
                    "f p n -> p f n"))
                xt_t[(b, tt)] = xt_all

            def p1(b, tt):
                """Token tile tt of batch b: qkv projection from prefetched xT."""
                if tt == 0:
                    qT_t[b] = qkv_pool.tile([128, N], BF16, tag="qT", name=f"qT{b}")
                    kT_t[b] = qkv_pool.tile([128, N], BF16, tag="kT", name=f"kT{b}")
                    v_t[b] = qkv_pool.tile([128, NJ, VW], BF16, tag="v",
                                           name=f"v{b}")
                    nc.vector.memset(v_t[b][:, :, DH::72], 1.0)
                qT, kT, v_sb = qT_t[b], kT_t[b], v_t[b]
                xt_all = xt_t.pop((b, tt))
                xt = [xt_all[:, ft, :] for ft in range(FT)]
                vts = vt_pool.tile([128, 512], BF16, tag="vt")
                ts_ = slice(tt * 512, (tt + 1) * 512)
                for blk, dest in ((0, qT[:, ts_]), (1, kT[:, ts_]),
                                  (2, vts[:])):
                    pp = ps_work.tile([128, 512], F32, tag="work")
                    for ft in range(FT):
                        nc.tensor.matmul(
                            pp[:], w_sb[:, ft, blk * 128:(blk + 1) * 128],
                            xt[ft],
                            start=(ft == 0), stop=(ft == FT - 1))
                    nc.vector.tensor_copy(dest, pp[:])
                for sub in range(4):
                    pv = ps_work.tile([128, 512], F32, tag="work")
                    nc.tensor.matmul(
                        pv[:, 0:128], vts[:, sub * 128:(sub + 1) * 128],
                        ident[:], start=True, stop=True)
                    ch = tt * 4 + sub
                    nc.vector.tensor_copy(v_sb[:, ch, 0:DH], pv[:, 0:DH])
                    nc.vector.tensor_copy(v_sb[:, ch, 72:72 + DH],
                                          pv[:, DH:2 * DH])

            def p2(b, ni):
                """Attention for n_i tile ni of batch b."""
                if ni == 0:
                    ostk_t[b] = ostk_pool.tile([128, N], BF16, tag="ostk",
                                               name=f"ostk{b}")
                qT, kT, v_sb, ostk = qT_t[b], kT_t[b], v_t[b], ostk_t[b]
                pavA = ps_av.tile([128, 512], F32, tag="av")
                pavB = ps_av.tile([128, 512], F32, tag="av")
                for nj in range(NJ):
                    ps = ps_score.tile([128, 1024], F32, tag="score")
                    kcol = slice(nj * 128, (nj + 1) * 128)
                    qcol = slice(ni * 512, (ni + 1) * 512)
                    nc.tensor.matmul(ps[:, 0:512], kT[0:DH, kcol],
                                     qT[0:DH, qcol], start=True, stop=True)
                    nc.tensor.matmul(ps[:, 512:1024], kT[DH:128, kcol],
                                     qT[DH:128, qcol], start=True, stop=True)
                    at = attn_pool.tile([128, 1024], BF16, tag="attn")
                    nc.scalar.activation(at[:], ps[:], EXP, scale=0.125)
                    nc.tensor.matmul(
                        pavA[0:DH + 1, :], v_sb[:, nj, 0:DH + 1],
                        at[:, 0:512],
                        start=(nj == 0), stop=(nj == NJ - 1))
                    nc.tensor.matmul(
                        pavB[0:DH + 1, :], v_sb[:, nj, 72:72 + DH + 1],
                        at[:, 512:1024],
                        start=(nj == 0), stop=(nj == NJ - 1))
                # evacuate accumulators fast (keeps PE fed), then normalize
                # off the critical path (reciprocal + GpSimd broadcast + mul)
                ocols = slice(ni * 512, (ni + 1) * 512)
                ovA = ov_pool.tile([DH, 512], F32, tag="ov")
                nc.vector.tensor_copy(ovA[:], pavA[0:DH, :])
                ovB = ov_pool.tile([DH, 512], F32, tag="ov")
                nc.vector.tensor_copy(ovB[:], pavB[0:DH, :])
                srow = smol_pool.tile([1, 1024], F32, tag="srow")
                nc.vector.tensor_copy(srow[0:1, 0:512], pavA[DH:DH + 1, :])
                nc.vector.tensor_copy(srow[0:1, 512:1024], pavB[DH:DH + 1, :])
                rcp = smol_pool.tile([1, 1024], F32, tag="rcp")
                nc.vector.reciprocal_approx_fast(rcp[:], srow[:])
                rbA = smol_pool.tile([DH, 512], F32, tag="rbA")
                nc.gpsimd.partition_broadcast(rbA[:], rcp[0:1, 0:512])
                rbB = smol_pool.tile([DH, 512], F32, tag="rbB")
                nc.gpsimd.partition_broadcast(rbB[:], rcp[0:1, 512:1024])
                nc.vector.tensor_mul(ostk[0:DH, ocols], rbA[:], ovA[:])
                nc.vector.tensor_mul(ostk[DH:128, ocols], rbB[:], ovB[:])

            def p3(b, g):
                """Output projection for token chunks 4g..4g+3 of batch b."""
                ostk = ostk_t[b]
                fo = fout_pool.tile([128, 4, D], F32, tag="fout")
                for ch in range(4):
                    tc_ = 4 * g + ch
                    for half in range(2):
                        pf = ps_work.tile([128, 512], F32, tag="work")
                        nc.tensor.matmul(
                            pf[:], ostk[:, tc_ * 128:(tc_ + 1) * 128],
                            wout_sb[:, half * 512:(half + 1) * 512],
                            start=True, stop=True)
                        nc.vector.tensor_copy(
                            fo[:, ch, half * 512:(half + 1) * 512], pf[:])
                base = b * N + 4 * g * 128
                nc.sync.dma_start(
                    out_d[base:base + 512, :].rearrange("(c p) m -> p c m", p=128),
                    fo[:])

            # software pipeline: P1(0) | P2(b) x P1(b+1) x P3(b-1) | P3(3)
            # xT loads are issued one step ahead of the projections.
            for tt in range(TT_PER_B):
                p1_load(0, tt)
            for tt in range(TT_PER_B):
                p1(0, tt)
            for b in range(B):
                for i in range(4):
                    if b + 1 < B:
                        if i == 0:
                            p1_load(b + 1, 0)
                            p1_load(b + 1, 1)
                        elif i + 1 < 4:
                            p1_load(b + 1, i + 1)
                    p2(b, i)
                    if b + 1 < B:
                        p1(b + 1, i)
                    if b >= 1:
                        p3(b - 1, i)
                    if b == B - 1:
                        p3(b, i)

    nc.compile()
    return nc


def make_in_maps(x, w_qkv, w_out):
    xT_bf = np.ascontiguousarray(x.reshape(NT, D).T).astype(ml_dtypes.bfloat16)
    in_maps = []
    for c in range(8):
        cols = slice(c * 128, (c + 1) * 128)
        w_local = np.concatenate(
            [w_qkv[:, o * HEADS * DH:][:, cols] for o in range(3)], axis=1)
        in_maps.append({
            "xT": xT_bf,
            "wqkv": np.ascontiguousarray(w_local).astype(ml_dtypes.bfloat16),
            "wout": np.ascontiguousarray(w_out[c * 128:(c + 1) * 128, :]).astype(
                ml_dtypes.bfloat16),
        })
    return in_maps


def kernel(x, w_qkv, w_out):
    x = np.asarray(x, dtype=np.float32)
    w_qkv = np.asarray(w_qkv, dtype=np.float32)
    w_out = np.asarray(w_out, dtype=np.float32)
    if "nc" not in _CACHE:
        _CACHE["nc"] = build()
    nc = _CACHE["nc"]

    res = run_bass_kernel_spmd(nc, make_in_maps(x, w_qkv, w_out),
                               core_ids=list(range(8)))
    total = res.results[0]["out"]
    for c in range(1, 8):
        total = total + res.results[c]["out"]
    return total.reshape(B, N, D).astype(np.float32)


# revision 36
# speedup vs baseline: 1.1922x; 1.1922x over previous
"""Multi-head attention block (dense transformer) on 8 Trainium2 NeuronCores.

Problem: x [4, 2048, 1024] f32, w_qkv [1024, 3072], w_out [1024, 1024].
  qkv = x @ w_qkv -> split (3, 16 heads, 64) -> softmax(q k^T / 8) v -> @ w_out

Sharding: tensor-parallel over heads. Core c owns heads (2c, 2c+1):
  - w_qkv columns for q/k/v of those heads -> [1024, 384]
  - w_out rows for those heads            -> [128, 1024]
  - x is pre-transposed/cast on the host to xT [1024, 8192] bf16 (the
    contraction side must sit on partitions; doing it host-side avoids 512
    on-chip PE transposes per core)
  - each core computes a full-shape partial output [8192, 1024]; the host sum
    of the 8 partials is the all-reduce.

Per-core kernel (all matmuls bf16 into fp32 PSUM):
  P1: project qT,kT,vT [128=2*64 rows, n] (scoresT-friendly layout) from xT
      tiles; PE-transpose vT back to v natural [n, 128] stored with a ones
      column per head (softmax sums).
  P2: per (batch, n_i tile of 512, n_j chunk of 128): both heads' scoresT
      [n_j=128, 512] go into one PSUM tile so the two K=64 score matmuls are
      co-ready and adjacent -> the PE row-group-packs them into one
      concurrent stream pass; one ACT exp per chunk (scale=1/8 folded in,
      no max-subtraction needed: scores ~ N(0,1)) -> bf16; av matmul
      lhsT=[v|1] (M=65) accumulates outT [64, 512] + softmax sums in row 64.
      Accumulators are evacuated to SBUF immediately (frees PSUM, keeps PE
      fed); normalization (fast reciprocal of both heads' sums + GpSimd
      partition_broadcast + DVE multiply) runs off the critical path
      -> out_stack [128, n].
  P3: partial = out_stack.T @ w_out_local, streamed to DRAM in 4-chunk
      batched stores.

The three phases are software-pipelined across batches (P2(b) interleaved
with P1(b+1) and P3(b-1)) so the PE always has dense independent work and
the HAM clock gate stays at K=8/8. PSUM budget is exactly 8 banks:
2 work (P1/P3) + 2x2 score + 2 av.
"""

import numpy as np
import ml_dtypes

import concourse.bacc as bacc
import concourse.tile as tile
from concourse import mybir, masks
from concourse.bass_utils import run_bass_kernel_spmd

F32 = mybir.dt.float32
BF16 = mybir.dt.bfloat16
EXP = mybir.ActivationFunctionType.Exp

B = 4
N = 2048
D = 1024
HEADS = 16
DH = 64
NT = B * N           # 8192 tokens
FT = D // 128        # 8 feature chunks
TT_PER_B = 4         # token tiles (512) per batch
NI = 4               # n_i tiles of 512 per batch
NJ = 16              # n_j chunks of 128 per batch
VW = 144             # v chunk: [v_A(64) | 1 | pad7 | v_B(64) | 1 | pad] (16B-aligned)

_CACHE = {}


def build():
    nc = bacc.Bacc("TRN2", target_bir_lowering=False, debug=False, num_devices=1)
    xT_d = nc.dram_tensor("xT", [D, NT], BF16, kind="ExternalInput").ap()
    wqkv_d = nc.dram_tensor("wqkv", [D, 384], BF16, kind="ExternalInput").ap()
    wout_d = nc.dram_tensor("wout", [128, D], BF16, kind="ExternalInput").ap()
    out_d = nc.dram_tensor("out", [NT, D], F32, kind="ExternalOutput").ap()
    xT_v = xT_d.rearrange("(f p) n -> f p n", p=128)

    with tile.TileContext(nc) as tc:
        with tc.tile_pool(name="const", bufs=1) as cpool, \
             tc.tile_pool(name="xt", bufs=4) as xt_pool, \
             tc.tile_pool(name="qkv", bufs=2) as qkv_pool, \
             tc.tile_pool(name="vt", bufs=3) as vt_pool, \
             tc.tile_pool(name="attn", bufs=4) as attn_pool, \
             tc.tile_pool(name="ostk", bufs=2) as ostk_pool, \
             tc.tile_pool(name="ov", bufs=4) as ov_pool, \
             tc.tile_pool(name="smol", bufs=6) as smol_pool, \
             tc.tile_pool(name="fout", bufs=2) as fout_pool, \
             tc.tile_pool(name="ps_work", bufs=2, space="PSUM") as ps_work, \
             tc.tile_pool(name="ps_score", bufs=2, space="PSUM") as ps_score, \
             tc.tile_pool(name="ps_av", bufs=2, space="PSUM") as ps_av:

            ident = cpool.tile([128, 128], BF16, tag="ident")
            masks.make_identity(nc, ident[:])

            wv = wqkv_d.rearrange("(f p) m -> p f m", p=128)
            w_sb = cpool.tile([128, FT, 384], BF16, tag="w")
            nc.sync.dma_start(w_sb[:, 0:4, :], wv[:, 0:4, :])
            nc.sync.dma_start(w_sb[:, 4:8, :], wv[:, 4:8, :])
            wout_sb = cpool.tile([128, D], BF16, tag="wout")
            nc.sync.dma_start(wout_sb[:], wout_d)

            # per-batch live tiles
            qT_t, kT_t, v_t, ostk_t, xt_t = {}, {}, {}, {}, {}

            def p1_load(b, tt):
                """Prefetch the xT chunk for token tile tt of batch b."""
                tok = slice(b * N + tt * 512, b * N + (tt + 1) * 512)
                xt_all = xt_pool.tile([128, FT, 512], BF16, tag="xt",
                                      name=f"xt{b}_{tt}")
                nc.sync.dma_start(xt_all[:], xT_v[:, :, tok].rearrange(
                    "f p n -> p f n"))
                xt_t[(b, tt)] = xt_all

            def p1(b, tt):
                """Token tile tt of batch b: qkv projection from prefetched xT."""
                if tt == 0:
                    qT_t[b] = qkv_pool.tile([128, N], BF16, tag="qT", name=f"qT{b}")
                    kT_t[b] = qkv_pool.tile([128, N], BF16, tag="kT", name=f"kT{b}")
                    v_t[b] = qkv_pool.tile([128, NJ, VW], BF16, tag="v",
                                           name=f"v{b}")
                    nc.vector.memset(v_t[b][:, :, DH::72], 1.0)
                qT, kT, v_sb = qT_t[b], kT_t[b], v_t[b]
                xt_all = xt_t.pop((b, tt))
                xt = [xt_all[:, ft, :] for ft in range(FT)]
                vts = vt_pool.tile([128, 512], BF16, tag="vt")
                ts_ = slice(tt * 512, (tt + 1) * 512)
                for blk, dest in ((0, qT[:, ts_]), (1, kT[:, ts_]),
                                  (2, vts[:])):
                    pp = ps_work.tile([128, 512], F32, tag="work")
                    for ft in range(FT):
                        nc.tensor.matmul(
                            pp[:], w_sb[:, ft, blk * 128:(blk + 1) * 128],
                            xt[ft],
                            start=(ft == 0), stop=(ft == FT - 1))
                    nc.vector.tensor_copy(dest, pp[:])
                for sub in range(4):
                    pv = ps_work.tile([128, 512], F32, tag="work")
                    nc.tensor.matmul(
                        pv[:, 0:128], vts[:, sub * 128:(sub + 1) * 128],
                        ident[:], start=True, stop=True)
                    ch = tt * 4 + sub
                    nc.vector.tensor_copy(v_sb[:, ch, 0:DH], pv[:, 0:DH])
                    nc.vector.tensor_copy(v_sb[:, ch, 72:72 + DH],
                                          pv[:, DH:2 * DH])

            def p2(b, ni):
                """Attention for n_i tile ni of batch b."""
                if ni == 0:
                    ostk_t[b] = ostk_pool.tile([128, N], BF16, tag="ostk",
                                               name=f"ostk{b}")
                qT, kT, v_sb, ostk = qT_t[b], kT_t[b], v_t[b], ostk_t[b]
                pavA = ps_av.tile([128, 512], F32, tag="av")
                pavB = ps_av.tile([128, 512], F32, tag="av")
                for nj in range(NJ):
                    ps = ps_score.tile([128, 1024], F32, tag="score")
                    kcol = slice(nj * 128, (nj + 1) * 128)
                    qcol = slice(ni * 512, (ni + 1) * 512)
                    nc.tensor.matmul(ps[:, 0:512], kT[0:DH, kcol],
                                     qT[0:DH, qcol], start=True, stop=True)
                    nc.tensor.matmul(ps[:, 512:1024], kT[DH:128, kcol],
                                     qT[DH:128, qcol], start=True, stop=True)
                    at = attn_pool.tile([128, 1024], BF16, tag="attn")
                    nc.scalar.activation(at[:], ps[:], EXP, scale=0.125)
                    nc.tensor.matmul(
                        pavA[0:DH + 1, :], v_sb[:, nj, 0:DH + 1],
                        at[:, 0:512],
                        start=(nj == 0), stop=(nj == NJ - 1))
                    nc.tensor.matmul(
                        pavB[0:DH + 1, :], v_sb[:, nj, 72:72 + DH + 1],
                        at[:, 512:1024],
                        start=(nj == 0), stop=(nj == NJ - 1))
                # evacuate accumulators fast (keeps PE fed), then normalize
                # off the critical path (reciprocal + GpSimd broadcast + mul)
                ocols = slice(ni * 512, (ni + 1) * 512)
                ovA = ov_pool.tile([DH, 512], F32, tag="ov")
                nc.vector.tensor_copy(ovA[:], pavA[0:DH, :])
                ovB = ov_pool.tile([DH, 512], F32, tag="ov")
                nc.vector.tensor_copy(ovB[:], pavB[0:DH, :])
                srow = smol_pool.tile([1, 1024], F32, tag="srow")
                nc.vector.tensor_copy(srow[0:1, 0:512], pavA[DH:DH + 1, :])
                nc.vector.tensor_copy(srow[0:1, 512:1024], pavB[DH:DH + 1, :])
                rcp = smol_pool.tile([1, 1024], F32, tag="rcp")
                nc.vector.reciprocal_approx_fast(rcp[:], srow[:])
                rbA = smol_pool.tile([DH, 512], F32, tag="rbA")
                nc.gpsimd.partition_broadcast(rbA[:], rcp[0:1, 0:512])
                rbB = smol_pool.tile([DH, 512], F32, tag="rbB")
                nc.gpsimd.partition_broadcast(rbB[:], rcp[0:1, 512:1024])
                nc.vector.tensor_mul(ostk[0:DH, ocols], rbA[:], ovA[:])
                nc.vector.tensor_mul(ostk[DH:128, ocols], rbB[:], ovB[:])

            def p3(b, g):
                """Output projection for token chunks 4g..4g+3 of batch b."""
                ostk = ostk_t[b]
                fo = fout_pool.tile([128, 4, D], F32, tag="fout")
                for ch in range(4):
                    tc_ = 4 * g + ch
                    for half in range(2):
                        pf = ps_work.tile([128, 512], F32, tag="work")
                        nc.tensor.matmul(
                            pf[:], ostk[:, tc_ * 128:(tc_ + 1) * 128],
                            wout_sb[:, half * 512:(half + 1) * 512],
                            start=True, stop=True)
                        nc.vector.tensor_copy(
                            fo[:, ch, half * 512:(half + 1) * 512], pf[:])
                base = b * N + 4 * g * 128
                nc.sync.dma_start(
                    out_d[base:base + 512, :].rearrange("(c p) m -> p c m", p=128),
                    fo[:])

            # software pipeline: P1(0) | P2(b) x P1(b+1) x P3(b-1) | P3(3)
            # xT loads are issued one step ahead of the projections.
            for tt in range(TT_PER_B):
                p1_load(0, tt)
                p1(0, tt)
            for b in range(B):
                for i in range(4):
                    if b + 1 < B:
                        p1_load(b + 1, i)
                    p2(b, i)
                    if b + 1 < B:
                        p1(b + 1, i)
                    if b >= 1:
                        p3(b - 1, i)
                    if b == B - 1:
                        p3(b, i)

    nc.compile()
    return nc


def make_in_maps(x, w_qkv, w_out):
    xT_bf = np.ascontiguousarray(x.reshape(NT, D).T).astype(ml_dtypes.bfloat16)
    in_maps = []
    for c in range(8):
        cols = slice(c * 128, (c + 1) * 128)
        w_local = np.concatenate(
            [w_qkv[:, o * HEADS * DH:][:, cols] for o in range(3)], axis=1)
        in_maps.append({
            "xT": xT_bf,
            "wqkv": np.ascontiguousarray(w_local).astype(ml_dtypes.bfloat16),
            "wout": np.ascontiguousarray(w_out[c * 128:(c + 1) * 128, :]).astype(
                ml_dtypes.bfloat16),
        })
    return in_maps


def kernel(x, w_qkv, w_out):
    x = np.asarray(x, dtype=np.float32)
    w_qkv = np.asarray(w_qkv, dtype=np.float32)
    w_out = np.asarray(w_out, dtype=np.float32)
    if "nc" not in _CACHE:
        _CACHE["nc"] = build()
    nc = _CACHE["nc"]

    res = run_bass_kernel_spmd(nc, make_in_maps(x, w_qkv, w_out),
                               core_ids=list(range(8)))
    total = res.results[0]["out"]
    for c in range(1, 8):
        total = total + res.results[c]["out"]
    return total.reshape(B, N, D).astype(np.float32)


# revision 37
# speedup vs baseline: 1.1943x; 1.0018x over previous
"""Multi-head attention block (dense transformer) on 8 Trainium2 NeuronCores.

Problem: x [4, 2048, 1024] f32, w_qkv [1024, 3072], w_out [1024, 1024].
  qkv = x @ w_qkv -> split (3, 16 heads, 64) -> softmax(q k^T / 8) v -> @ w_out

Sharding: tensor-parallel over heads. Core c owns heads (2c, 2c+1):
  - w_qkv columns for q/k/v of those heads -> [1024, 384]
  - w_out rows for those heads            -> [128, 1024]
  - x is pre-transposed/cast on the host to xT [1024, 8192] bf16 (the
    contraction side must sit on partitions; doing it host-side avoids 512
    on-chip PE transposes per core)
  - each core computes a full-shape partial output [8192, 1024]; the host sum
    of the 8 partials is the all-reduce.

Per-core kernel (all matmuls bf16 into fp32 PSUM):
  P1: project qT,kT,vT [128=2*64 rows, n] (scoresT-friendly layout) from xT
      tiles; PE-transpose vT back to v natural [n, 128] stored with a ones
      column per head (softmax sums).
  P2: per (batch, n_i tile of 512, n_j chunk of 128): both heads' scoresT
      [n_j=128, 512] go into one PSUM tile so the two K=64 score matmuls are
      co-ready and adjacent -> the PE row-group-packs them into one
      concurrent stream pass; one ACT exp per chunk (scale=1/8 folded in,
      no max-subtraction needed: scores ~ N(0,1)) -> bf16; av matmul
      lhsT=[v|1] (M=65) accumulates outT [64, 512] + softmax sums in row 64.
      Accumulators are evacuated to SBUF immediately (frees PSUM, keeps PE
      fed); normalization (fast reciprocal of both heads' sums + GpSimd
      partition_broadcast + DVE multiply) runs off the critical path
      -> out_stack [128, n].
  P3: partial = out_stack.T @ w_out_local, streamed to DRAM in 4-chunk
      batched stores.

The three phases are software-pipelined across batches (P2(b) interleaved
with P1(b+1) and P3(b-1)) so the PE always has dense independent work and
the HAM clock gate stays at K=8/8. PSUM budget is exactly 8 banks:
2 work (P1/P3) + 2x2 score + 2 av.
"""

import numpy as np
import ml_dtypes

import concourse.bacc as bacc
import concourse.tile as tile
from concourse import mybir, masks
from concourse.bass_utils import run_bass_kernel_spmd

F32 = mybir.dt.float32
BF16 = mybir.dt.bfloat16
EXP = mybir.ActivationFunctionType.Exp

B = 4
N = 2048
D = 1024
HEADS = 16
DH = 64
NT = B * N           # 8192 tokens
FT = D // 128        # 8 feature chunks
TT_PER_B = 4         # token tiles (512) per batch
NI = 4               # n_i tiles of 512 per batch
NJ = 16              # n_j chunks of 128 per batch
VW = 144             # v chunk: [v_A(64) | 1 | pad7 | v_B(64) | 1 | pad] (16B-aligned)

_CACHE = {}


def build():
    nc = bacc.Bacc("TRN2", target_bir_lowering=False, debug=False, num_devices=1)
    xT_d = nc.dram_tensor("xT", [D, NT], BF16, kind="ExternalInput").ap()
    wqkv_d = nc.dram_tensor("wqkv", [D, 384], BF16, kind="ExternalInput").ap()
    wout_d = nc.dram_tensor("wout", [128, D], BF16, kind="ExternalInput").ap()
    out_d = nc.dram_tensor("out", [NT, D], F32, kind="ExternalOutput").ap()
    xT_v = xT_d.rearrange("(f p) n -> f p n", p=128)

    with tile.TileContext(nc) as tc:
        with tc.tile_pool(name="const", bufs=1) as cpool, \
             tc.tile_pool(name="xt", bufs=4) as xt_pool, \
             tc.tile_pool(name="qkv", bufs=2) as qkv_pool, \
             tc.tile_pool(name="vt", bufs=3) as vt_pool, \
             tc.tile_pool(name="attn", bufs=4) as attn_pool, \
             tc.tile_pool(name="ostk", bufs=2) as ostk_pool, \
             tc.tile_pool(name="ov", bufs=4) as ov_pool, \
             tc.tile_pool(name="smol", bufs=6) as smol_pool, \
             tc.tile_pool(name="fout", bufs=2) as fout_pool, \
             tc.tile_pool(name="ps_work", bufs=2, space="PSUM") as ps_work, \
             tc.tile_pool(name="ps_score", bufs=2, space="PSUM") as ps_score, \
             tc.tile_pool(name="ps_av", bufs=2, space="PSUM") as ps_av:

            ident = cpool.tile([128, 128], BF16, tag="ident")
            masks.make_identity(nc, ident[:])

            wv = wqkv_d.rearrange("(f p) m -> p f m", p=128)
            w_sb = cpool.tile([128, FT, 384], BF16, tag="w")
            nc.gpsimd.dma_start(w_sb[:, 0:4, :], wv[:, 0:4, :])
            nc.gpsimd.dma_start(w_sb[:, 4:8, :], wv[:, 4:8, :])
            wout_sb = cpool.tile([128, D], BF16, tag="wout")
            nc.gpsimd.dma_start(wout_sb[:], wout_d)

            # per-batch live tiles
            qT_t, kT_t, v_t, ostk_t, xt_t = {}, {}, {}, {}, {}

            def p1_load(b, tt):
                """Prefetch the xT chunk for token tile tt of batch b."""
                tok = slice(b * N + tt * 512, b * N + (tt + 1) * 512)
                xt_all = xt_pool.tile([128, FT, 512], BF16, tag="xt",
                                      name=f"xt{b}_{tt}")
                nc.sync.dma_start(xt_all[:], xT_v[:, :, tok].rearrange(
                    "f p n -> p f n"))
                xt_t[(b, tt)] = xt_all

            def p1(b, tt):
                """Token tile tt of batch b: qkv projection from prefetched xT."""
                if tt == 0:
                    qT_t[b] = qkv_pool.tile([128, N], BF16, tag="qT", name=f"qT{b}")
                    kT_t[b] = qkv_pool.tile([128, N], BF16, tag="kT", name=f"kT{b}")
                    v_t[b] = qkv_pool.tile([128, NJ, VW], BF16, tag="v",
                                           name=f"v{b}")
                    nc.vector.memset(v_t[b][:, :, DH::72], 1.0)
                qT, kT, v_sb = qT_t[b], kT_t[b], v_t[b]
                xt_all = xt_t.pop((b, tt))
                xt = [xt_all[:, ft, :] for ft in range(FT)]
                vts = vt_pool.tile([128, 512], BF16, tag="vt")
                ts_ = slice(tt * 512, (tt + 1) * 512)
                for blk, dest in ((0, qT[:, ts_]), (1, kT[:, ts_]),
                                  (2, vts[:])):
                    pp = ps_work.tile([128, 512], F32, tag="work")
                    for ft in range(FT):
                        nc.tensor.matmul(
                            pp[:], w_sb[:, ft, blk * 128:(blk + 1) * 128],
                            xt[ft],
                            start=(ft == 0), stop=(ft == FT - 1))
                    nc.vector.tensor_copy(dest, pp[:])
                for sub in range(4):
                    pv = ps_work.tile([128, 512], F32, tag="work")
                    nc.tensor.matmul(
                        pv[:, 0:128], vts[:, sub * 128:(sub + 1) * 128],
                        ident[:], start=True, stop=True)
                    ch = tt * 4 + sub
                    nc.vector.tensor_copy(v_sb[:, ch, 0:DH], pv[:, 0:DH])
                    nc.vector.tensor_copy(v_sb[:, ch, 72:72 + DH],
                                          pv[:, DH:2 * DH])

            def p2(b, ni):
                """Attention for n_i tile ni of batch b."""
                if ni == 0:
                    ostk_t[b] = ostk_pool.tile([128, N], BF16, tag="ostk",
                                               name=f"ostk{b}")
                qT, kT, v_sb, ostk = qT_t[b], kT_t[b], v_t[b], ostk_t[b]
                pavA = ps_av.tile([128, 512], F32, tag="av")
                pavB = ps_av.tile([128, 512], F32, tag="av")
                for nj in range(NJ):
                    ps = ps_score.tile([128, 1024], F32, tag="score")
                    kcol = slice(nj * 128, (nj + 1) * 128)
                    qcol = slice(ni * 512, (ni + 1) * 512)
                    nc.tensor.matmul(ps[:, 0:512], kT[0:DH, kcol],
                                     qT[0:DH, qcol], start=True, stop=True)
                    nc.tensor.matmul(ps[:, 512:1024], kT[DH:128, kcol],
                                     qT[DH:128, qcol], start=True, stop=True)
                    at = attn_pool.tile([128, 1024], BF16, tag="attn")
                    nc.scalar.activation(at[:], ps[:], EXP, scale=0.125)
                    nc.tensor.matmul(
                        pavA[0:DH + 1, :], v_sb[:, nj, 0:DH + 1],
                        at[:, 0:512],
                        start=(nj == 0), stop=(nj == NJ - 1))
                    nc.tensor.matmul(
                        pavB[0:DH + 1, :], v_sb[:, nj, 72:72 + DH + 1],
                        at[:, 512:1024],
                        start=(nj == 0), stop=(nj == NJ - 1))
                # evacuate accumulators fast (keeps PE fed), then normalize
                # off the critical path (reciprocal + GpSimd broadcast + mul)
                ocols = slice(ni * 512, (ni + 1) * 512)
                ovA = ov_pool.tile([DH, 512], F32, tag="ov")
                nc.vector.tensor_copy(ovA[:], pavA[0:DH, :])
                ovB = ov_pool.tile([DH, 512], F32, tag="ov")
                nc.vector.tensor_copy(ovB[:], pavB[0:DH, :])
                srow = smol_pool.tile([1, 1024], F32, tag="srow")
                nc.vector.tensor_copy(srow[0:1, 0:512], pavA[DH:DH + 1, :])
                nc.vector.tensor_copy(srow[0:1, 512:1024], pavB[DH:DH + 1, :])
                rcp = smol_pool.tile([1, 1024], F32, tag="rcp")
                nc.vector.reciprocal_approx_fast(rcp[:], srow[:])
                rbA = smol_pool.tile([DH, 512], F32, tag="rbA")
                nc.gpsimd.partition_broadcast(rbA[:], rcp[0:1, 0:512])
                rbB = smol_pool.tile([DH, 512], F32, tag="rbB")
                nc.gpsimd.partition_broadcast(rbB[:], rcp[0:1, 512:1024])
                nc.vector.tensor_mul(ostk[0:DH, ocols], rbA[:], ovA[:])
                nc.vector.tensor_mul(ostk[DH:128, ocols], rbB[:], ovB[:])

            def p3(b, g):
                """Output projection for token chunks 4g..4g+3 of batch b."""
                ostk = ostk_t[b]
                fo = fout_pool.tile([128, 4, D], F32, tag="fout")
                for ch in range(4):
                    tc_ = 4 * g + ch
                    for half in range(2):
                        pf = ps_work.tile([128, 512], F32, tag="work")
                        nc.tensor.matmul(
                            pf[:], ostk[:, tc_ * 128:(tc_ + 1) * 128],
                            wout_sb[:, half * 512:(half + 1) * 512],
                            start=True, stop=True)
                        nc.vector.tensor_copy(
                            fo[:, ch, half * 512:(half + 1) * 512], pf[:])
                base = b * N + 4 * g * 128
                nc.gpsimd.dma_start(
                    out_d[base:base + 512, :].rearrange("(c p) m -> p c m", p=128),
                    fo[:])

            # software pipeline: P1(0) | P2(b) x P1(b+1) x P3(b-1) | P3(3)
            # xT loads are issued one step ahead of the projections.
            for tt in range(TT_PER_B):
                p1_load(0, tt)
                p1(0, tt)
            for b in range(B):
                for i in range(4):
                    if b + 1 < B:
                        p1_load(b + 1, i)
                    p2(b, i)
                    if b + 1 < B:
                        p1(b + 1, i)
                    if b >= 1:
                        p3(b - 1, i)
                    if b == B - 1:
                        p3(b, i)

    nc.compile()
    return nc


def make_in_maps(x, w_qkv, w_out):
    xT_bf = np.ascontiguousarray(x.reshape(NT, D).T).astype(ml_dtypes.bfloat16)
    in_maps = []
    for c in range(8):
        cols = slice(c * 128, (c + 1) * 128)
        w_local = np.concatenate(
            [w_qkv[:, o * HEADS * DH:][:, cols] for o in range(3)], axis=1)
        in_maps.append({
            "xT": xT_bf,
            "wqkv": np.ascontiguousarray(w_local).astype(ml_dtypes.bfloat16),
            "wout": np.ascontiguousarray(w_out[c * 128:(c + 1) * 128, :]).astype(
                ml_dtypes.bfloat16),
        })
    return in_maps


def kernel(x, w_qkv, w_out):
    x = np.asarray(x, dtype=np.float32)
    w_qkv = np.asarray(w_qkv, dtype=np.float32)
    w_out = np.asarray(w_out, dtype=np.float32)
    if "nc" not in _CACHE:
        _CACHE["nc"] = build()
    nc = _CACHE["nc"]

    res = run_bass_kernel_spmd(nc, make_in_maps(x, w_qkv, w_out),
                               core_ids=list(range(8)))
    total = res.results[0]["out"]
    for c in range(1, 8):
        total = total + res.results[c]["out"]
    return total.reshape(B, N, D).astype(np.float32)


# revision 38
# speedup vs baseline: 1.1963x; 1.0017x over previous
"""Multi-head attention block (dense transformer) on 8 Trainium2 NeuronCores.

Problem: x [4, 2048, 1024] f32, w_qkv [1024, 3072], w_out [1024, 1024].
  qkv = x @ w_qkv -> split (3, 16 heads, 64) -> softmax(q k^T / 8) v -> @ w_out

Sharding: tensor-parallel over heads. Core c owns heads (2c, 2c+1):
  - w_qkv columns for q/k/v of those heads -> [1024, 384]
  - w_out rows for those heads            -> [128, 1024]
  - x is pre-transposed/cast on the host to xT [1024, 8192] bf16 (the
    contraction side must sit on partitions; doing it host-side avoids 512
    on-chip PE transposes per core)
  - each core computes a full-shape partial output [8192, 1024]; the host sum
    of the 8 partials is the all-reduce.

Per-core kernel (all matmuls bf16 into fp32 PSUM):
  P1: project qT,kT,vT [128=2*64 rows, n] (scoresT-friendly layout) from xT
      tiles; PE-transpose vT back to v natural [n, 128] stored with a ones
      column per head (softmax sums).
  P2: per (batch, n_i tile of 512, n_j chunk of 128): both heads' scoresT
      [n_j=128, 512] go into one PSUM tile so the two K=64 score matmuls are
      co-ready and adjacent -> the PE row-group-packs them into one
      concurrent stream pass; one ACT exp per chunk (scale=1/8 folded in,
      no max-subtraction needed: scores ~ N(0,1)) -> bf16; av matmul
      lhsT=[v|1] (M=65) accumulates outT [64, 512] + softmax sums in row 64.
      Accumulators are evacuated to SBUF immediately (frees PSUM, keeps PE
      fed); normalization (fast reciprocal of both heads' sums + GpSimd
      partition_broadcast + DVE multiply) runs off the critical path
      -> out_stack [128, n].
  P3: partial = out_stack.T @ w_out_local, streamed to DRAM in 4-chunk
      batched stores.

The three phases are software-pipelined across batches (P2(b) interleaved
with P1(b+1) and P3(b-1)) so the PE always has dense independent work and
the HAM clock gate stays at K=8/8. PSUM budget is exactly 8 banks:
2 work (P1/P3) + 2x2 score + 2 av.
"""

import numpy as np
import ml_dtypes

import concourse.bacc as bacc
import concourse.tile as tile
from concourse import mybir, masks
from concourse.bass_utils import run_bass_kernel_spmd

F32 = mybir.dt.float32
BF16 = mybir.dt.bfloat16
EXP = mybir.ActivationFunctionType.Exp

B = 4
N = 2048
D = 1024
HEADS = 16
DH = 64
NT = B * N           # 8192 tokens
FT = D // 128        # 8 feature chunks
TT_PER_B = 4         # token tiles (512) per batch
NI = 4               # n_i tiles of 512 per batch
NJ = 16              # n_j chunks of 128 per batch
VW = 144             # v chunk: [v_A(64) | 1 | pad7 | v_B(64) | 1 | pad] (16B-aligned)

_CACHE = {}


def build():
    nc = bacc.Bacc("TRN2", target_bir_lowering=False, debug=False, num_devices=1)
    xT_d = nc.dram_tensor("xT", [D, NT], BF16, kind="ExternalInput").ap()
    wqkv_d = nc.dram_tensor("wqkv", [D, 384], BF16, kind="ExternalInput").ap()
    wout_d = nc.dram_tensor("wout", [128, D], BF16, kind="ExternalInput").ap()
    out_d = nc.dram_tensor("out", [NT, D], F32, kind="ExternalOutput").ap()
    xT_v = xT_d.rearrange("(f p) n -> f p n", p=128)

    with tile.TileContext(nc) as tc:
        with tc.tile_pool(name="const", bufs=1) as cpool, \
             tc.tile_pool(name="xt", bufs=4) as xt_pool, \
             tc.tile_pool(name="qkv", bufs=2) as qkv_pool, \
             tc.tile_pool(name="vt", bufs=3) as vt_pool, \
             tc.tile_pool(name="attn", bufs=4) as attn_pool, \
             tc.tile_pool(name="ostk", bufs=2) as ostk_pool, \
             tc.tile_pool(name="ov", bufs=4) as ov_pool, \
             tc.tile_pool(name="smol", bufs=6) as smol_pool, \
             tc.tile_pool(name="fout", bufs=2) as fout_pool, \
             tc.tile_pool(name="ps_work", bufs=2, space="PSUM") as ps_work, \
             tc.tile_pool(name="ps_score", bufs=2, space="PSUM") as ps_score, \
             tc.tile_pool(name="ps_av", bufs=2, space="PSUM") as ps_av:

            # weight DMAs first so nothing queues ahead of them
            wv = wqkv_d.rearrange("(f p) m -> p f m", p=128)
            w_sb = cpool.tile([128, FT, 384], BF16, tag="w")
            nc.gpsimd.dma_start(w_sb[:, 0:4, :], wv[:, 0:4, :])
            nc.gpsimd.dma_start(w_sb[:, 4:8, :], wv[:, 4:8, :])
            wout_sb = cpool.tile([128, D], BF16, tag="wout")
            nc.gpsimd.dma_start(wout_sb[:], wout_d)
            ident = cpool.tile([128, 128], BF16, tag="ident")
            masks.make_identity(nc, ident[:])

            # per-batch live tiles
            qT_t, kT_t, v_t, ostk_t, xt_t = {}, {}, {}, {}, {}

            def p1_load(b, tt):
                """Prefetch the xT chunk for token tile tt of batch b."""
                tok = slice(b * N + tt * 512, b * N + (tt + 1) * 512)
                xt_all = xt_pool.tile([128, FT, 512], BF16, tag="xt",
                                      name=f"xt{b}_{tt}")
                nc.sync.dma_start(xt_all[:], xT_v[:, :, tok].rearrange(
                    "f p n -> p f n"))
                xt_t[(b, tt)] = xt_all

            def p1(b, tt):
                """Token tile tt of batch b: qkv projection from prefetched xT."""
                if tt == 0:
                    qT_t[b] = qkv_pool.tile([128, N], BF16, tag="qT", name=f"qT{b}")
                    kT_t[b] = qkv_pool.tile([128, N], BF16, tag="kT", name=f"kT{b}")
                    v_t[b] = qkv_pool.tile([128, NJ, VW], BF16, tag="v",
                                           name=f"v{b}")
                    nc.vector.memset(v_t[b][:, :, DH::72], 1.0)
                qT, kT, v_sb = qT_t[b], kT_t[b], v_t[b]
                xt_all = xt_t.pop((b, tt))
                xt = [xt_all[:, ft, :] for ft in range(FT)]
                vts = vt_pool.tile([128, 512], BF16, tag="vt")
                ts_ = slice(tt * 512, (tt + 1) * 512)
                for blk, dest in ((0, qT[:, ts_]), (1, kT[:, ts_]),
                                  (2, vts[:])):
                    pp = ps_work.tile([128, 512], F32, tag="work")
                    for ft in range(FT):
                        nc.tensor.matmul(
                            pp[:], w_sb[:, ft, blk * 128:(blk + 1) * 128],
                            xt[ft],
                            start=(ft == 0), stop=(ft == FT - 1))
                    nc.vector.tensor_copy(dest, pp[:])
                for sub in range(4):
                    pv = ps_work.tile([128, 512], F32, tag="work")
                    nc.tensor.matmul(
                        pv[:, 0:128], vts[:, sub * 128:(sub + 1) * 128],
                        ident[:], start=True, stop=True)
                    ch = tt * 4 + sub
                    nc.vector.tensor_copy(v_sb[:, ch, 0:DH], pv[:, 0:DH])
                    nc.vector.tensor_copy(v_sb[:, ch, 72:72 + DH],
                                          pv[:, DH:2 * DH])

            def p2(b, ni):
                """Attention for n_i tile ni of batch b."""
                if ni == 0:
                    ostk_t[b] = ostk_pool.tile([128, N], BF16, tag="ostk",
                                               name=f"ostk{b}")
                qT, kT, v_sb, ostk = qT_t[b], kT_t[b], v_t[b], ostk_t[b]
                pavA = ps_av.tile([128, 512], F32, tag="av")
                pavB = ps_av.tile([128, 512], F32, tag="av")
                for nj in range(NJ):
                    ps = ps_score.tile([128, 1024], F32, tag="score")
                    kcol = slice(nj * 128, (nj + 1) * 128)
                    qcol = slice(ni * 512, (ni + 1) * 512)
                    nc.tensor.matmul(ps[:, 0:512], kT[0:DH, kcol],
                                     qT[0:DH, qcol], start=True, stop=True)
                    nc.tensor.matmul(ps[:, 512:1024], kT[DH:128, kcol],
                                     qT[DH:128, qcol], start=True, stop=True)
                    at = attn_pool.tile([128, 1024], BF16, tag="attn")
                    nc.scalar.activation(at[:], ps[:], EXP, scale=0.125)
                    nc.tensor.matmul(
                        pavA[0:DH + 1, :], v_sb[:, nj, 0:DH + 1],
                        at[:, 0:512],
                        start=(nj == 0), stop=(nj == NJ - 1))
                    nc.tensor.matmul(
                        pavB[0:DH + 1, :], v_sb[:, nj, 72:72 + DH + 1],
                        at[:, 512:1024],
                        start=(nj == 0), stop=(nj == NJ - 1))
                # evacuate accumulators fast (keeps PE fed), then normalize
                # off the critical path (reciprocal + GpSimd broadcast + mul)
                ocols = slice(ni * 512, (ni + 1) * 512)
                ovA = ov_pool.tile([DH, 512], F32, tag="ov")
                nc.vector.tensor_copy(ovA[:], pavA[0:DH, :])
                ovB = ov_pool.tile([DH, 512], F32, tag="ov")
                nc.vector.tensor_copy(ovB[:], pavB[0:DH, :])
                srow = smol_pool.tile([1, 1024], F32, tag="srow")
                nc.vector.tensor_copy(srow[0:1, 0:512], pavA[DH:DH + 1, :])
                nc.vector.tensor_copy(srow[0:1, 512:1024], pavB[DH:DH + 1, :])
                rcp = smol_pool.tile([1, 1024], F32, tag="rcp")
                nc.vector.reciprocal_approx_fast(rcp[:], srow[:])
                rbA = smol_pool.tile([DH, 512], F32, tag="rbA")
                nc.gpsimd.partition_broadcast(rbA[:], rcp[0:1, 0:512])
                rbB = smol_pool.tile([DH, 512], F32, tag="rbB")
                nc.gpsimd.partition_broadcast(rbB[:], rcp[0:1, 512:1024])
                nc.vector.tensor_mul(ostk[0:DH, ocols], rbA[:], ovA[:])
                nc.vector.tensor_mul(ostk[DH:128, ocols], rbB[:], ovB[:])

            def p3(b, g):
                """Output projection for token chunks 4g..4g+3 of batch b."""
                ostk = ostk_t[b]
                fo = fout_pool.tile([128, 4, D], F32, tag="fout")
                for ch in range(4):
                    tc_ = 4 * g + ch
                    for half in range(2):
                        pf = ps_work.tile([128, 512], F32, tag="work")
                        nc.tensor.matmul(
                            pf[:], ostk[:, tc_ * 128:(tc_ + 1) * 128],
                            wout_sb[:, half * 512:(half + 1) * 512],
                            start=True, stop=True)
                        nc.vector.tensor_copy(
                            fo[:, ch, half * 512:(half + 1) * 512], pf[:])
                base = b * N + 4 * g * 128
                nc.gpsimd.dma_start(
                    out_d[base:base + 512, :].rearrange("(c p) m -> p c m", p=128),
                    fo[:])

            # software pipeline: P1(0) | P2(b) x P1(b+1) x P3(b-1) | P3(3)
            # xT loads are issued one step ahead of the projections.
            for tt in range(TT_PER_B):
                p1_load(0, tt)
                p1(0, tt)
            for b in range(B):
                for i in range(4):
                    if b + 1 < B:
                        p1_load(b + 1, i)
                    p2(b, i)
                    if b + 1 < B:
                        p1(b + 1, i)
                    if b >= 1:
                        p3(b - 1, i)
                    if b == B - 1:
                        p3(b, i)

    nc.compile()
    return nc


def make_in_maps(x, w_qkv, w_out):
    xT_bf = np.ascontiguousarray(x.reshape(NT, D).T).astype(ml_dtypes.bfloat16)
    in_maps = []
    for c in range(8):
        cols = slice(c * 128, (c + 1) * 128)
        w_local = np.concatenate(
            [w_qkv[:, o * HEADS * DH:][:, cols] for o in range(3)], axis=1)
        in_maps.append({
            "xT": xT_bf,
            "wqkv": np.ascontiguousarray(w_local).astype(ml_dtypes.bfloat16),
            "wout": np.ascontiguousarray(w_out[c * 128:(c + 1) * 128, :]).astype(
                ml_dtypes.bfloat16),
        })
    return in_maps


def kernel(x, w_qkv, w_out):
    x = np.asarray(x, dtype=np.float32)
    w_qkv = np.asarray(w_qkv, dtype=np.float32)
    w_out = np.asarray(w_out, dtype=np.float32)
    if "nc" not in _CACHE:
        _CACHE["nc"] = build()
    nc = _CACHE["nc"]

    res = run_bass_kernel_spmd(nc, make_in_maps(x, w_qkv, w_out),
                               core_ids=list(range(8)))
    total = res.results[0]["out"]
    for c in range(1, 8):
        total = total + res.results[c]["out"]
    return total.reshape(B, N, D).astype(np.float32)


# revision 39
# speedup vs baseline: 1.2115x; 1.0127x over previous
"""Multi-head attention block (dense transformer) on 8 Trainium2 NeuronCores.

Problem: x [4, 2048, 1024] f32, w_qkv [1024, 3072], w_out [1024, 1024].
  qkv = x @ w_qkv -> split (3, 16 heads, 64) -> softmax(q k^T / 8) v -> @ w_out

Sharding: tensor-parallel over heads. Core c owns heads (2c, 2c+1):
  - w_qkv columns for q/k/v of those heads -> [1024, 384]
  - w_out rows for those heads            -> [128, 1024]
  - x is pre-transposed/cast on the host to xT [1024, 8192] bf16 (the
    contraction side must sit on partitions; doing it host-side avoids 512
    on-chip PE transposes per core)
  - each core computes a full-shape partial output [8192, 1024]; the host sum
    of the 8 partials is the all-reduce.

Per-core kernel (all matmuls bf16 into fp32 PSUM):
  P1: project qT,kT,vT [128=2*64 rows, n] (scoresT-friendly layout) from xT
      tiles; PE-transpose vT back to v natural [n, 128] stored with a ones
      column per head (softmax sums).
  P2: per (batch, n_i tile of 512, n_j chunk of 128): both heads' scoresT
      [n_j=128, 512] go into one PSUM tile so the two K=64 score matmuls are
      co-ready and adjacent -> the PE row-group-packs them into one
      concurrent stream pass; one ACT exp per chunk (scale=1/8 folded in,
      no max-subtraction needed: scores ~ N(0,1)) -> bf16; av matmul
      lhsT=[v|1] (M=65) accumulates outT [64, 512] + softmax sums in row 64.
      Accumulators are evacuated to SBUF immediately (frees PSUM, keeps PE
      fed); normalization (fast reciprocal of both heads' sums + GpSimd
      partition_broadcast + DVE multiply) runs off the critical path
      -> out_stack [128, n].
  P3: partial = out_stack.T @ w_out_local, streamed to DRAM in 4-chunk
      batched stores.

The three phases are software-pipelined across batches (P2(b) interleaved
with P1(b+1) and P3(b-1)) so the PE always has dense independent work and
the HAM clock gate stays at K=8/8. PSUM budget is exactly 8 banks:
2 work (P1/P3) + 2x2 score + 2 av.
"""

import numpy as np
import ml_dtypes

import concourse.bacc as bacc
import concourse.tile as tile
from concourse import mybir, masks
from concourse.bass_utils import run_bass_kernel_spmd

F32 = mybir.dt.float32
BF16 = mybir.dt.bfloat16
EXP = mybir.ActivationFunctionType.Exp

B = 4
N = 2048
D = 1024
HEADS = 16
DH = 64
NT = B * N           # 8192 tokens
FT = D // 128        # 8 feature chunks
TT_PER_B = 4         # token tiles (512) per batch
NI = 4               # n_i tiles of 512 per batch
NJ = 16              # n_j chunks of 128 per batch
VW = 144             # v chunk: [v_A(64) | 1 | pad7 | v_B(64) | 1 | pad] (16B-aligned)

_CACHE = {}


def build():
    nc = bacc.Bacc("TRN2", target_bir_lowering=False, debug=False, num_devices=1)
    xT_d = nc.dram_tensor("xT", [D, NT], BF16, kind="ExternalInput").ap()
    wqkv_d = nc.dram_tensor("wqkv", [D, 384], BF16, kind="ExternalInput").ap()
    wout_d = nc.dram_tensor("wout", [128, D], BF16, kind="ExternalInput").ap()
    out_d = nc.dram_tensor("out", [NT, D], F32, kind="ExternalOutput").ap()
    xT_v = xT_d.rearrange("(f p) n -> f p n", p=128)

    with tile.TileContext(nc) as tc:
        with tc.tile_pool(name="const", bufs=1) as cpool, \
             tc.tile_pool(name="xt", bufs=4) as xt_pool, \
             tc.tile_pool(name="qkv", bufs=2) as qkv_pool, \
             tc.tile_pool(name="vt", bufs=3) as vt_pool, \
             tc.tile_pool(name="attn", bufs=4) as attn_pool, \
             tc.tile_pool(name="ostk", bufs=2) as ostk_pool, \
             tc.tile_pool(name="ov", bufs=4) as ov_pool, \
             tc.tile_pool(name="smol", bufs=6) as smol_pool, \
             tc.tile_pool(name="fout", bufs=2) as fout_pool, \
             tc.tile_pool(name="ps_work", bufs=2, space="PSUM") as ps_work, \
             tc.tile_pool(name="ps_score", bufs=2, space="PSUM") as ps_score, \
             tc.tile_pool(name="ps_av", bufs=2, space="PSUM") as ps_av:

            # weight DMAs first so nothing queues ahead of them
            wv = wqkv_d.rearrange("(f p) m -> p f m", p=128)
            w_sb = cpool.tile([128, FT, 384], BF16, tag="w")
            nc.gpsimd.dma_start(w_sb[:, 0:4, :], wv[:, 0:4, :])
            nc.gpsimd.dma_start(w_sb[:, 4:8, :], wv[:, 4:8, :])
            wout_sb = cpool.tile([128, D], BF16, tag="wout")
            nc.gpsimd.dma_start(wout_sb[:], wout_d)
            ident = cpool.tile([128, 128], BF16, tag="ident")
            masks.make_identity(nc, ident[:])

            # per-batch live tiles
            qT_t, kT_t, v_t, ostk_t, xt_t = {}, {}, {}, {}, {}

            def p1_load(b, tt):
                """Prefetch the xT chunk for token tile tt of batch b."""
                tok = slice(b * N + tt * 512, b * N + (tt + 1) * 512)
                xt_all = xt_pool.tile([128, FT, 512], BF16, tag="xt",
                                      name=f"xt{b}_{tt}")
                nc.sync.dma_start(xt_all[:], xT_v[:, :, tok].rearrange(
                    "f p n -> p f n"))
                xt_t[(b, tt)] = xt_all

            def p1(b, tt):
                """Token tile tt of batch b: qkv projection from prefetched xT."""
                if tt == 0:
                    qT_t[b] = qkv_pool.tile([128, N], BF16, tag="qT", name=f"qT{b}")
                    kT_t[b] = qkv_pool.tile([128, N], BF16, tag="kT", name=f"kT{b}")
                    v_t[b] = qkv_pool.tile([128, NJ, VW], BF16, tag="v",
                                           name=f"v{b}")
                    nc.vector.memset(v_t[b][:, :, DH::72], 1.0)
                qT, kT, v_sb = qT_t[b], kT_t[b], v_t[b]
                xt_all = xt_t.pop((b, tt))
                xt = [xt_all[:, ft, :] for ft in range(FT)]
                vts = vt_pool.tile([128, 512], BF16, tag="vt")
                ts_ = slice(tt * 512, (tt + 1) * 512)
                for blk, dest in ((2, vts[:]), (0, qT[:, ts_]),
                                  (1, kT[:, ts_])):
                    pp = ps_work.tile([128, 512], F32, tag="work")
                    for ft in range(FT):
                        nc.tensor.matmul(
                            pp[:], w_sb[:, ft, blk * 128:(blk + 1) * 128],
                            xt[ft],
                            start=(ft == 0), stop=(ft == FT - 1))
                    nc.vector.tensor_copy(dest, pp[:])
                for sub in range(4):
                    pv = ps_work.tile([128, 512], F32, tag="work")
                    nc.tensor.matmul(
                        pv[:, 0:128], vts[:, sub * 128:(sub + 1) * 128],
                        ident[:], start=True, stop=True)
                    ch = tt * 4 + sub
                    nc.vector.tensor_copy(v_sb[:, ch, 0:DH], pv[:, 0:DH])
                    nc.vector.tensor_copy(v_sb[:, ch, 72:72 + DH],
                                          pv[:, DH:2 * DH])

            def p2(b, ni):
                """Attention for n_i tile ni of batch b."""
                if ni == 0:
                    ostk_t[b] = ostk_pool.tile([128, N], BF16, tag="ostk",
                                               name=f"ostk{b}")
                qT, kT, v_sb, ostk = qT_t[b], kT_t[b], v_t[b], ostk_t[b]
                pavA = ps_av.tile([128, 512], F32, tag="av")
                pavB = ps_av.tile([128, 512], F32, tag="av")
                for nj in range(NJ):
                    ps = ps_score.tile([128, 1024], F32, tag="score")
                    kcol = slice(nj * 128, (nj + 1) * 128)
                    qcol = slice(ni * 512, (ni + 1) * 512)
                    nc.tensor.matmul(ps[:, 0:512], kT[0:DH, kcol],
                                     qT[0:DH, qcol], start=True, stop=True)
                    nc.tensor.matmul(ps[:, 512:1024], kT[DH:128, kcol],
                                     qT[DH:128, qcol], start=True, stop=True)
                    at = attn_pool.tile([128, 1024], BF16, tag="attn")
                    nc.scalar.activation(at[:], ps[:], EXP, scale=0.125)
                    nc.tensor.matmul(
                        pavA[0:DH + 1, :], v_sb[:, nj, 0:DH + 1],
                        at[:, 0:512],
                        start=(nj == 0), stop=(nj == NJ - 1))
                    nc.tensor.matmul(
                        pavB[0:DH + 1, :], v_sb[:, nj, 72:72 + DH + 1],
                        at[:, 512:1024],
                        start=(nj == 0), stop=(nj == NJ - 1))
                # evacuate accumulators fast (keeps PE fed), then normalize
                # off the critical path (reciprocal + GpSimd broadcast + mul)
                ocols = slice(ni * 512, (ni + 1) * 512)
                ovA = ov_pool.tile([DH, 512], F32, tag="ov")
                nc.vector.tensor_copy(ovA[:], pavA[0:DH, :])
                ovB = ov_pool.tile([DH, 512], F32, tag="ov")
                nc.vector.tensor_copy(ovB[:], pavB[0:DH, :])
                srow = smol_pool.tile([1, 1024], F32, tag="srow")
                nc.vector.tensor_copy(srow[0:1, 0:512], pavA[DH:DH + 1, :])
                nc.vector.tensor_copy(srow[0:1, 512:1024], pavB[DH:DH + 1, :])
                rcp = smol_pool.tile([1, 1024], F32, tag="rcp")
                nc.vector.reciprocal_approx_fast(rcp[:], srow[:])
                rbA = smol_pool.tile([DH, 512], F32, tag="rbA")
                nc.gpsimd.partition_broadcast(rbA[:], rcp[0:1, 0:512])
                rbB = smol_pool.tile([DH, 512], F32, tag="rbB")
                nc.gpsimd.partition_broadcast(rbB[:], rcp[0:1, 512:1024])
                nc.vector.tensor_mul(ostk[0:DH, ocols], rbA[:], ovA[:])
                nc.vector.tensor_mul(ostk[DH:128, ocols], rbB[:], ovB[:])

            def p3(b, g):
                """Output projection for token chunks 4g..4g+3 of batch b."""
                ostk = ostk_t[b]
                fo = fout_pool.tile([128, 4, D], F32, tag="fout")
                for ch in range(4):
                    tc_ = 4 * g + ch
                    for half in range(2):
                        pf = ps_work.tile([128, 512], F32, tag="work")
                        nc.tensor.matmul(
                            pf[:], ostk[:, tc_ * 128:(tc_ + 1) * 128],
                            wout_sb[:, half * 512:(half + 1) * 512],
                            start=True, stop=True)
                        nc.vector.tensor_copy(
                            fo[:, ch, half * 512:(half + 1) * 512], pf[:])
                base = b * N + 4 * g * 128
                nc.gpsimd.dma_start(
                    out_d[base:base + 512, :].rearrange("(c p) m -> p c m", p=128),
                    fo[:])

            # software pipeline: P1(0) | P2(b) x P1(b+1) x P3(b-1) | P3(3)
            # xT loads are issued one step ahead of the projections.
            for tt in range(TT_PER_B):
                p1_load(0, tt)
                p1(0, tt)
            for b in range(B):
                for i in range(4):
                    if b + 1 < B:
                        p1_load(b + 1, i)
                    p2(b, i)
                    if b + 1 < B:
                        p1(b + 1, i)
                    if b >= 1:
                        p3(b - 1, i)
                    if b == B - 1:
                        p3(b, i)

    nc.compile()
    return nc


def make_in_maps(x, w_qkv, w_out):
    xT_bf = np.ascontiguousarray(x.reshape(NT, D).T).astype(ml_dtypes.bfloat16)
    in_maps = []
    for c in range(8):
        cols = slice(c * 128, (c + 1) * 128)
        w_local = np.concatenate(
            [w_qkv[:, o * HEADS * DH:][:, cols] for o in range(3)], axis=1)
        in_maps.append({
            "xT": xT_bf,
            "wqkv": np.ascontiguousarray(w_local).astype(ml_dtypes.bfloat16),
            "wout": np.ascontiguousarray(w_out[c * 128:(c + 1) * 128, :]).astype(
                ml_dtypes.bfloat16),
        })
    return in_maps


def kernel(x, w_qkv, w_out):
    x = np.asarray(x, dtype=np.float32)
    w_qkv = np.asarray(w_qkv, dtype=np.float32)
    w_out = np.asarray(w_out, dtype=np.float32)
    if "nc" not in _CACHE:
        _CACHE["nc"] = build()
    nc = _CACHE["nc"]

    res = run_bass_kernel_spmd(nc, make_in_maps(x, w_qkv, w_out),
                               core_ids=list(range(8)))
    total = res.results[0]["out"]
    for c in range(1, 8):
        total = total + res.results[c]["out"]
    return total.reshape(B, N, D).astype(np.float32)
